# revision 26
# baseline (speedup 1.0000x reference)
"""DeeperGCN forward pass on 8 Trainium2 NeuronCores (Bass/Tile).

Strategy (graph-parallel, per sharding hint):
  - Nodes are binned into 8 cores x 20 chunks x 128 slots (125 real nodes per
    bin, degree-balanced via snake round-robin) so every chunk has ~2000
    incoming edges; edges live with their destination core.
  - Per GENConv layer each core computes em=exp(t*msg), me=msg*em for its own
    nodes, AllGathers the packed fp16 table, then gathers table rows for its
    edges with dma_gather and segment-sums them with one-hot matmuls into
    PSUM.  agg = sum(me)/(sum(em)+1e-16) reproduces the softmax aggregation.
  - Dense MLP (w1 -> LayerNorm -> relu -> w2) runs node-major per 128-node
    chunk with PE transposes; graph LayerNorm uses a 2-float AllReduce;
    global_add_pool is a one-hot matmul followed by a [64,256] AllReduce.
"""

import numpy as np

N_NODES = 20000
N_EDGES = 320000
F_IN = 128
H = 256
H2 = 512
OUT_F = 64
NUM_G = 64
NCORES = 8
C = 20            # chunks per core
BPC = 128         # node slots per chunk
NPB = 125         # real nodes per chunk (20000 / 160)
ROWS = C * BPC    # 2560 node slots per core
GROWS = NCORES * ROWS
EPS_MSG = 1e-7
LN_EPS = 1e-5
NTOT = float(N_NODES * H)


def _preprocess(x, edge_index, batch):
    """Bin nodes, build per-core gather indices / one-hot tiles."""
    src = edge_index[0].astype(np.int64)
    dst = edge_index[1].astype(np.int64)
    deg = np.bincount(dst, minlength=N_NODES)

    # snake round-robin of degree-sorted nodes over 160 bins -> 125 nodes/bin
    order = np.argsort(-deg, kind="stable")
    nbins = NCORES * C
    node_core = np.empty(N_NODES, np.int64)
    node_chunk = np.empty(N_NODES, np.int64)
    node_pos = np.empty(N_NODES, np.int64)
    rounds = N_NODES // nbins  # 125
    idx = np.arange(N_NODES)
    rnd = idx // nbins
    pos_in_round = idx % nbins
    binidx = np.where(rnd % 2 == 0, pos_in_round, nbins - 1 - pos_in_round)
    node_core[order] = binidx % NCORES
    node_chunk[order] = binidx // NCORES
    node_pos[order] = rnd
    assert rounds == NPB

    # slot id of every node in the AllGather'ed table
    slot = node_core * ROWS + node_chunk * BPC + node_pos  # < 20480
    assert slot.max() < 2 ** 15

    # tiles per chunk
    e_core = node_core[dst]
    e_chunk = node_chunk[dst]
    bin_of_edge = e_core * C + e_chunk
    bin_edges = np.bincount(bin_of_edge, minlength=nbins)
    T = int(np.ceil(bin_edges.max() / 128))
    EPC = T * 128

    # x fed pre-transposed: xT[c] = [F_IN, ROWS] so encoder needs no PE transpose
    x_sh = np.zeros((NCORES, ROWS, F_IN), np.float32)
    rows_all = node_chunk * BPC + node_pos
    for c in range(NCORES):
        m = node_core == c
        x_sh[c, rows_all[m]] = x[m]
    x_t = np.ascontiguousarray(x_sh.transpose(0, 2, 1))  # [NC, F_IN, ROWS]

    # per-node graph id (for on-device pool one-hot); -1 for empty slots
    batchpos = np.full((NCORES, ROWS), -1.0, np.float32)
    b = batch.astype(np.int64)
    for c in range(NCORES):
        m = node_core == c
        batchpos[c, rows_all[m]] = b[m].astype(np.float32)
    # [NC, 128, C] layout: batchpos[c][p, k] = graph of node (chunk k, pos p)
    batchpos = np.ascontiguousarray(
        batchpos.reshape(NCORES, C, BPC).transpose(0, 2, 1))

    # per (core, chunk): edge src slots + one-hot dst columns
    src_slots = np.zeros((NCORES, C, EPC), np.int16)
    dst_cols = np.full((NCORES, C, EPC), -1, np.int64)
    order_e = np.lexsort((dst, bin_of_edge))
    s_sorted = src[order_e]
    d_sorted = dst[order_e]
    bin_sorted = bin_of_edge[order_e]
    starts = np.searchsorted(bin_sorted, np.arange(nbins))
    ends = np.searchsorted(bin_sorted, np.arange(nbins) + 1)
    for bi in range(nbins):
        c, k = bi // C, bi % C
        lo, hi = starts[bi], ends[bi]
        n = hi - lo
        src_slots[c, k, :n] = slot[s_sorted[lo:hi]].astype(np.int16)
        dst_cols[c, k, :n] = node_pos[d_sorted[lo:hi]]

    # wrapped int16 index layout: element i of a chunk at [i % 16, i // 16];
    # fed as 16 partitions, replicated to 128 on device
    src_idx = (
        src_slots.reshape(NCORES, C, EPC // 16, 16)
        .transpose(0, 1, 3, 2)            # [NC, C, 16, EPC//16]
        .transpose(0, 2, 1, 3)            # [NC, 16, C, EPC//16]
        .reshape(NCORES, 16, C * EPC // 16).copy()
    )

    # packed dst position per edge for on-device one-hot generation:
    # dstc[c][p, k*T + t] = dst position (0..127) of edge t*128+p of chunk k
    dstc = (dst_cols.astype(np.float32)
            .reshape(NCORES, C, T, 128)
            .transpose(0, 3, 1, 2)        # [NC, 128, C, T]
            .reshape(NCORES, 128, C * T)
            .astype(np.float16).copy())

    return x_t, src_idx, dstc, batchpos, T


def _build_program(T, flags, weights, repeat=1):
    import concourse.bacc as bacc
    import concourse.bass as bass
    import concourse.mybir as mybir
    import concourse.tile as tile
    from concourse import library_config

    f32 = mybir.dt.float32
    bf16 = mybir.dt.bfloat16
    f16 = mybir.dt.float16
    i16 = mybir.dt.int16
    AF = mybir.ActivationFunctionType
    ALU = mybir.AluOpType
    EPC = T * 128
    RG = [list(range(NCORES))]

    import os
    _skip_coll = bool(int(os.environ.get("K_SKIP_COLL", "0")))
    _skip_gath = bool(int(os.environ.get("K_SKIP_GATH", "0")))
    _nq = int(os.environ.get("K_NQUEUES", "4"))
    _shared_ag = bool(int(os.environ.get("K_SHARED", "1")))
    nc = bacc.Bacc("TRN2", target_bir_lowering=False, debug=False,
                   num_devices=NCORES, num_swdge_queues=_nq)

    feeds = {}
    PER_CORE = {"x_t", "src_idx", "dstc", "batchpos"}

    def inp(name, arr, dtype):
        """Declare an input fed per-core (arr: [NCORES, ...]) or replicated."""
        feeds[name] = arr
        shape = list(arr.shape[1:]) if name in PER_CORE else list(arr.shape)
        return nc.dram_tensor(name, shape, dtype, kind="ExternalInput")

    x_in = inp("x_t", weights["x_t"], bf16)
    srcidx_d = inp("src_idx", weights["src_idx"], i16)
    dstc_d = inp("dstc", weights["dstc"], f16)
    batchpos_d = inp("batchpos", weights["batchpos"], f32)
    iota128_d = inp("iota128", weights["iota128"], f16)
    iota64_d = inp("iota64", weights["iota64"], f32)
    encw_d = inp("enc_w", weights["enc_w"], bf16)
    w1_d = [inp(f"w1_{l}", weights[f"w1_{l}"], bf16) for l in range(2)]
    w2_d = [inp(f"w2_{l}", weights[f"w2_{l}"], bf16) for l in range(2)]
    mw1_d = inp("mlp_w1", weights["mlp_w1"], bf16)
    mw2_d = inp("mlp_w2", weights["mlp_w2"], bf16)
    tvec_d = inp("t_vec", weights["t_vec"], f32)  # [128, 2]
    ident_d = inp("ident", weights["ident"], f32)  # [128, 128] identity
    out_d = nc.dram_tensor("out", [OUT_F, OUT_F], f32, kind="ExternalOutput")

    # optional generic-path tensors (replicated rows), only when non-trivial
    opt_d = {}
    for nm in flags:
        if flags[nm]:
            opt_d[nm] = inp(nm, weights[nm], f32)

    with tile.TileContext(nc) as tc:
        from contextlib import ExitStack

        with ExitStack() as ctx:
            cpool = ctx.enter_context(tc.tile_pool(name="const", bufs=1))
            wpool = ctx.enter_context(tc.tile_pool(name="wts", bufs=1))
            hpool = ctx.enter_context(tc.tile_pool(name="hbuf", bufs=1))
            sb = ctx.enter_context(tc.tile_pool(name="work", bufs=2))
            sb2 = ctx.enter_context(tc.tile_pool(name="work2", bufs=2))
            gpool = ctx.enter_context(tc.tile_pool(name="gather", bufs=7))
            empool = ctx.enter_context(tc.tile_pool(name="emme", bufs=2))
            ohpool = ctx.enter_context(tc.tile_pool(name="onehot", bufs=3))
            pk_pool = ctx.enter_context(tc.tile_pool(name="pack", bufs=2))
            ps_t = ctx.enter_context(
                tc.tile_pool(name="ps_t", bufs=2, space="PSUM"))
            ps_agg = ctx.enter_context(
                tc.tile_pool(name="ps_agg", bufs=3, space="PSUM"))
            ps_big = ctx.enter_context(
                tc.tile_pool(name="ps_big", bufs=2, space="PSUM"))
            ps_small = ctx.enter_context(
                tc.tile_pool(name="ps_small", bufs=1, space="PSUM"))
            dram = ctx.enter_context(
                tc.tile_pool(name="dram", bufs=1, space="DRAM"))

            # ---- constants & weights in SBUF ----
            nc.gpsimd.load_library(library_config.mlp)
            ident = cpool.tile([128, 128], f32)
            nc.sync.dma_start(ident[:], ident_d[:])
            ones_col = cpool.tile([128, 1], f32)
            nc.vector.memset(ones_col[:], 1.0)
            ones_row = cpool.tile([1, 128], f32)
            nc.vector.memset(ones_row[:], 1.0)
            epsln_col = cpool.tile([128, 1], f32)
            nc.vector.memset(epsln_col[:], LN_EPS)

            encw_sb = wpool.tile([128, H], bf16)
            nc.sync.dma_start(encw_sb[:], encw_d[:])
            w1_sb, w2_sb = [], []
            for l in range(2):
                a = wpool.tile([128, 2, H2], bf16, tag=f"w1sb{l}")
                nc.sync.dma_start(
                    a[:], w1_d[l].ap().rearrange("(j p) n -> p j n", p=128))
                w1_sb.append(a)
                bq = wpool.tile([128, 4, H], bf16, tag=f"w2sb{l}")
                nc.sync.dma_start(
                    bq[:], w2_d[l].ap().rearrange("(j p) n -> p j n", p=128))
                w2_sb.append(bq)
            mw1_sb = wpool.tile([128, 2, 128], bf16)
            nc.sync.dma_start(
                mw1_sb[:], mw1_d.ap().rearrange("(j p) n -> p j n", p=128))
            mw2_sb = wpool.tile([128, OUT_F], bf16)
            nc.sync.dma_start(mw2_sb[:], mw2_d[:])
            srcidx_sb = wpool.tile([128, C * EPC // 16], i16)
            for r in range(8):
                nc.sync.dma_start(srcidx_sb[r * 16:(r + 1) * 16, :], srcidx_d[:])
            xT_sb = wpool.tile([128, ROWS], bf16)
            nc.sync.dma_start(xT_sb[:], x_in[:])
            dstc_sb = wpool.tile([128, C * T], f16)
            nc.sync.dma_start(dstc_sb[:], dstc_d[:])
            batchpos_sb = wpool.tile([128, C], f32)
            nc.sync.dma_start(batchpos_sb[:], batchpos_d[:])
            iota128_sb = wpool.tile([128, 128], f16)
            nc.sync.dma_start(iota128_sb[:], iota128_d[:])
            iotaT_sb = wpool.tile([128, T, 128], f16)
            for _t in range(T):
                nc.vector.tensor_copy(iotaT_sb[:, _t, :], iota128_sb[:])
            iota64_sb = wpool.tile([128, NUM_G], f32)
            nc.sync.dma_start(iota64_sb[:], iota64_d[:])
            tvec_sb = wpool.tile([128, 2], f32)
            nc.sync.dma_start(tvec_sb[:], tvec_d[:])
            opt_sb = {}
            for nm, d in opt_d.items():
                tl = wpool.tile(list(d.shape), f32, tag=f"opt_{nm}")
                nc.sync.dma_start(tl[:], d[:])
                opt_sb[nm] = tl


            # ---- DRAM comm buffers (ag bufs per-rep: Shared DRAM wants a
            # single writer per buffer) ----
            _ag_space = "Shared" if _shared_ag else "Local"
            red_io = [dram.tile([1, 16], f32, tag=f"red{i}",
                                name=f"red{i}") for i in range(4)]
            pool_io = [dram.tile([OUT_F, H], f32, tag=f"pio{i}",
                                 name=f"pio{i}") for i in range(2)]

            from concourse.tile_rust import add_dep_helper

            def one_pass(prev_tail, rep):
                ag_in = [dram.tile([ROWS, H], f16, tag=f"ag_in{l}_r{rep}",
                                   name=f"ag_in{l}_r{rep}") for l in range(2)]
                ag_out = [dram.tile([GROWS, H], f16, tag=f"ag_out{l}_r{rep}",
                                    name=f"ag_out{l}_r{rep}",
                                    addr_space=_ag_space)
                          for l in range(2)]
                # per-rep activations (tags reuse slots across reps)
                h_enc = hpool.tile([128, C, H], f32, tag="h_enc",
                                   name="h_enc")
                h1 = hpool.tile([128, C, H], f32, tag="h1", name="h1")
                r_buf = hpool.tile([128, C, H], f32, tag="h_enc",
                                   name="r_buf")  # h_enc dead after conv0
                h2 = hpool.tile([128, C, H], f32, tag="h2", name="h2")
                # ================= encoder =================
                # table rows (raw h in fp16) stream to ag_in[0] as chunks
                # finish, so AllGather 0 can start right at encoder end
                for k in range(C):
                    hps = ps_big.tile([128, H2], f32, tag="z1")
                    _mm = nc.tensor.matmul(
                        hps[:, 0:H],
                        xT_sb[:, k * BPC:(k + 1) * BPC],
                        encw_sb[:],
                        start=True, stop=True)
                    if k == 0 and prev_tail is not None:
                        add_dep_helper(prev_tail.ins, _mm.ins, True,
                                       "serialize reps")
                    if flags.get("enc_b_rep"):
                        nc.vector.tensor_add(h_enc[:, k, :], hps[:, 0:H],
                                             opt_sb["enc_b_rep"][:])
                    else:
                        nc.any.tensor_copy(h_enc[:, k, :], hps[:, 0:H])
                    pk = pk_pool.tile([128, H], f16, tag="pk")
                    nc.any.tensor_copy(pk[:], h_enc[:, k, :])
                    nc.sync.dma_start(ag_in[0][k * BPC:(k + 1) * BPC, :], pk[:])

                # ================= helpers =================
                def issue_ag(l):
                    if not _skip_coll:
                        nc.gpsimd.collective_compute(
                            "AllGather", mybir.AluOpType.bypass,
                            replica_groups=RG,
                            ins=[ag_in[l].opt()], outs=[ag_out[l].opt()])
                    else:
                        nc.sync.dma_start(ag_out[l][0:ROWS, :], ag_in[l][:, :])

                def conv(l, x_t, res_t, out_t, ln_bc=None, next_agin=None,
                         next_stats=None):
                    """One GENConv layer. Table rows are RAW h values (fp16);
                    per gathered edge tile this computes
                      m  = relu((h - mu)*inv) + eps   (ln_bc) or relu(h)+eps
                      em = exp(t*m);  me = m*em
                    If next_agin is set, h_out rows stream to it (raw fp16)
                    as chunks finish; next_stats accumulates graph-LN stats.
                    """
                    TH = T // 2
                    for k in range(C):
                        emme = empool.tile([128, T, H2], f16, tag="emme",
                                           name="emme")
                        for h in range(2):
                            gt = gpool.tile([128, TH, H], f16, tag="gt",
                                            name="gt")
                            if not _skip_gath:
                                off = k * (EPC // 16) + h * (EPC // 32)
                                nc.gpsimd.dma_gather(
                                    gt[:], ag_out[l][:, :],
                                    srcidx_sb[:, off:off + EPC // 32],
                                    EPC // 2, EPC // 2, H,
                                    single_packet=False,
                                    queue_num=(2 * k + h) % _nq)
                            else:
                                nc.vector.memset(gt[:, 0, 0:4], 0.0)
                            if ln_bc is not None:
                                base = sb2.tile([128, TH, H], f16, tag="nrm")
                                nc.vector.tensor_scalar(
                                    base[:], gt[:], ln_bc[:, 0:1],
                                    ln_bc[:, 1:2], ALU.subtract, ALU.mult)
                                if flags.get(f"ng_rep_{l}"):
                                    nc.vector.tensor_mul(
                                        base[:], base[:],
                                        opt_sb[f"ng_rep_{l}"][:]
                                        .unsqueeze(1).broadcast_to([128, TH, H]))
                                if flags.get(f"nb_rep_{l}"):
                                    nc.vector.tensor_add(
                                        base[:], base[:],
                                        opt_sb[f"nb_rep_{l}"][:]
                                        .unsqueeze(1).broadcast_to([128, TH, H]))
                            else:
                                base = gt
                            mt = sb2.tile([128, TH, H], f16, tag="mt")
                            nc.vector.tensor_scalar(mt[:], base[:], 0.0,
                                                    EPS_MSG, ALU.max, ALU.add)
                            sl = emme[:, h * TH:(h + 1) * TH, :]
                            nc.scalar.activation(sl[:, :, 0:H], mt[:],
                                                 AF.Exp,
                                                 scale=tvec_sb[:, l:l + 1])
                            nc.vector.tensor_mul(sl[:, :, H:H2], mt[:],
                                                 sl[:, :, 0:H])
                        oh = ohpool.tile([128, T, 128], f16, tag="oh")
                        nc.vector.tensor_tensor(
                            oh[:], iotaT_sb[:],
                            dstc_sb[:, k * T:(k + 1) * T]
                            .unsqueeze(2).broadcast_to([128, T, 128]),
                            ALU.is_equal)
                        agg = ps_agg.tile([128, H2], f32, tag="agg")
                        for t in range(T):
                            nc.tensor.matmul(agg[:], oh[:, t, :],
                                             emme[:, t, :],
                                             start=(t == 0), stop=(t == T - 1))
                        den = sb.tile([128, H], f32, tag="den")
                        nc.vector.tensor_scalar_add(den[:], agg[:, 0:H], 1e-16)
                        rec = sb.tile([128, H], f32, tag="rec")
                        nc.vector.reciprocal(rec[:], den[:])
                        hin = sb.tile([128, H], f32, tag="hin")
                        nc.vector.tensor_mul(hin[:], agg[:, H:H2], rec[:])
                        nc.vector.tensor_add(hin[:], hin[:], x_t[:, k, :])
                        hinT = sb.tile([128, 2, 128], bf16, tag="hinT")
                        for j in range(2):
                            tp = ps_t.tile([128, 128], f32, tag="tp")
                            nc.tensor.transpose(
                                tp[:], hin[:, j * 128:(j + 1) * 128], ident[:])
                            nc.any.tensor_copy(hinT[:, j, :], tp[:])
                        z1 = ps_big.tile([128, H2], f32, tag="z1")
                        for j in range(2):
                            nc.tensor.matmul(z1[:], hinT[:, j, :],
                                             w1_sb[l][:, j, :],
                                             start=(j == 0), stop=(j == 1))
                        if flags.get(f"b1_rep_{l}"):
                            zb = sb2.tile([128, H2], f32, tag="zb")
                            nc.vector.tensor_add(zb[:], z1[:],
                                                 opt_sb[f"b1_rep_{l}"][:])
                            z1s = zb
                        else:
                            z1s = z1
                        st6 = sb.tile([128, 6], f32, tag="st6")
                        nc.vector.bn_stats(st6[:], z1s[:])
                        mv = sb.tile([128, 2], f32, tag="mv")
                        nc.vector.bn_aggr(mv[:], st6[:])
                        sd = sb.tile([128, 1], f32, tag="sd")
                        nc.scalar.activation(sd[:], mv[:, 1:2], AF.Sqrt,
                                             bias=epsln_col[:])
                        rs = sb.tile([128, 1], f32, tag="rs")
                        nc.vector.reciprocal(rs[:], sd[:])
                        h2c = sb2.tile([128, H2], f32, tag="h2c")
                        nc.vector.tensor_scalar(h2c[:], z1s[:], mv[:, 0:1], rs[:],
                                                ALU.subtract, ALU.mult)
                        if flags.get(f"lng_rep_{l}"):
                            nc.vector.tensor_mul(h2c[:], h2c[:],
                                                 opt_sb[f"lng_rep_{l}"][:])
                        if flags.get(f"lnb_rep_{l}"):
                            nc.vector.tensor_add(h2c[:], h2c[:],
                                                 opt_sb[f"lnb_rep_{l}"][:])
                        nc.scalar.activation(h2c[:], h2c[:], AF.Relu)
                        h2T = sb2.tile([128, 4, 128], bf16, tag="h2T")
                        for j in range(4):
                            tp = ps_t.tile([128, 128], f32, tag="tp")
                            nc.tensor.transpose(
                                tp[:], h2c[:, j * 128:(j + 1) * 128], ident[:])
                            nc.any.tensor_copy(h2T[:, j, :], tp[:])
                        ops = ps_big.tile([128, H2], f32, tag="z1")
                        for j in range(4):
                            nc.tensor.matmul(ops[:, 0:H], h2T[:, j, :],
                                             w2_sb[l][:, j, :],
                                             start=(j == 0), stop=(j == 3))
                        src_ap = ops[:, 0:H]
                        if flags.get(f"b2_rep_{l}"):
                            ob = sb.tile([128, H], f32, tag="ob")
                            nc.vector.tensor_add(ob[:], src_ap,
                                                 opt_sb[f"b2_rep_{l}"][:])
                            src_ap = ob[:]
                        if res_t is not None:
                            nc.vector.tensor_add(out_t[:, k, :], src_ap,
                                                 res_t[:, k, :])
                        else:
                            nc.any.tensor_copy(out_t[:, k, :], src_ap)
                        if next_agin is not None:
                            pk = pk_pool.tile([128, H], f16, tag="pk")
                            nc.any.tensor_copy(pk[:], out_t[:, k, :])
                            nc.sync.dma_start(
                                next_agin[k * BPC:(k + 1) * BPC, :], pk[:])
                        if next_stats is not None:
                            nc.vector.tensor_reduce(
                                next_stats[0:NPB, 0, k:k + 1],
                                out_t[0:NPB, k, :],
                                mybir.AxisListType.X, ALU.add)
                            scr = sb2.tile([128, H], f32, tag="scr")
                            nc.scalar.activation(
                                scr[0:NPB, :], out_t[0:NPB, k, :], AF.Square,
                                accum_out=next_stats[0:NPB, 1, k:k + 1])

                def ln_reduce(stats, io_base):
                    """stats [128,2,C] -> AllReduce -> bc [128,2] = [mu, inv]"""
                    cps = ps_t.tile([1, 2 * C], f32, tag="tp")
                    nc.tensor.matmul(cps[:], ones_col[0:NPB, :],
                                     stats[0:NPB, :, :],
                                     start=True, stop=True)
                    tot = sb.tile([1, 16], f32, tag="tot")
                    nc.vector.memset(tot[:], 0.0)
                    nc.vector.tensor_reduce(
                        tot[0:1, 0:2],
                        cps[0:1, :].rearrange("p (s k) -> p s k", k=C),
                        mybir.AxisListType.X, ALU.add)
                    nc.sync.dma_start(red_io[io_base][:], tot[:])
                    if not _skip_coll:
                        nc.gpsimd.collective_compute(
                            "AllReduce", ALU.add, replica_groups=RG,
                            ins=[red_io[io_base].opt()],
                            outs=[red_io[io_base + 1].opt()])
                    else:
                        nc.sync.dma_start(red_io[io_base + 1][:],
                                          red_io[io_base][:])
                    gt2 = sb.tile([1, 16], f32, tag="gt2")
                    nc.sync.dma_start(gt2[:], red_io[io_base + 1][:])
                    mu = sb.tile([1, 4], f32, tag="mu")
                    nc.vector.tensor_scalar_mul(mu[0:1, 0:2], gt2[0:1, 0:2],
                                                1.0 / NTOT)  # [mean, E[x^2]]
                    nc.vector.tensor_mul(mu[0:1, 2:3], mu[0:1, 0:1], mu[0:1, 0:1])
                    nc.vector.tensor_sub(mu[0:1, 2:3], mu[0:1, 1:2], mu[0:1, 2:3])
                    sdg = sb.tile([1, 1], f32, tag="sdg")
                    nc.scalar.activation(sdg[:], mu[0:1, 2:3], AF.Sqrt)
                    nc.vector.tensor_scalar_add(sdg[:], sdg[:], LN_EPS)
                    pair = sb.tile([1, 2], f32, tag="pair")
                    nc.any.tensor_copy(pair[0:1, 0:1], mu[0:1, 0:1])
                    nc.vector.reciprocal(pair[0:1, 1:2], sdg[:])
                    bcp = ps_t.tile([128, 2], f32, tag="tp")
                    nc.tensor.matmul(bcp[:], ones_row[:], pair[:],
                                     start=True, stop=True)
                    bc = sb.tile([128, 2], f32, tag="bc")
                    nc.any.tensor_copy(bc[:], bcp[:])
                    return bc

                def ln_apply(src_t, dst_t, bc, pref):
                    for k in range(C):
                        nc.vector.tensor_scalar(dst_t[:, k, :], src_t[:, k, :],
                                                bc[:, 0:1], bc[:, 1:2],
                                                ALU.subtract, ALU.mult)
                        if flags.get(f"ng_rep_{pref}"):
                            nc.vector.tensor_mul(dst_t[:, k, :], dst_t[:, k, :],
                                                 opt_sb[f"ng_rep_{pref}"][:])
                        if flags.get(f"nb_rep_{pref}"):
                            nc.vector.tensor_add(dst_t[:, k, :], dst_t[:, k, :],
                                                 opt_sb[f"nb_rep_{pref}"][:])
                        nc.scalar.activation(dst_t[:, k, :], dst_t[:, k, :],
                                             AF.Relu)

                # ================= network =================
                # chain: enc -> AG0 -> conv0 -> [AR(ln1) || AG1] -> conv1
                #        -> AR(ln0) -> pool -> AR(pool) -> MLP
                # conv0 streams raw h1 rows to ag_in[1] and accumulates ln1
                # stats as it goes; the ln1 AllReduce + bc + local r_buf all
                # hide behind AllGather 1.
                issue_ag(0)
                stats1 = sb2.tile([128, 2, C], f32, tag="stats",
                                  name="stats1")
                conv(0, h_enc, None, h1, next_agin=ag_in[1],
                     next_stats=stats1)
                bc1 = ln_reduce(stats1, 0)         # layers[1] norm scalars
                issue_ag(1)
                ln_apply(h1, r_buf, bc1, 1)        # local residual input
                stats0 = sb2.tile([128, 2, C], f32, tag="stats",
                                  name="stats0")
                conv(1, r_buf, h1, h2, ln_bc=bc1, next_stats=stats0)
                bc0 = ln_reduce(stats0, 2)         # layers[0] norm scalars
                ln_apply(h2, h2, bc0, 0)

                # ================= pool + final MLP =================
                plp = ps_small.tile([OUT_F, H], f32, tag="small")
                for k in range(C):
                    oh64 = sb.tile([128, NUM_G], f32, tag="oh64")
                    nc.vector.tensor_scalar(
                        oh64[:], iota64_sb[:], batchpos_sb[:, k:k + 1], None,
                        ALU.is_equal)
                    nc.tensor.matmul(plp[:], oh64[:], h2[:, k, :],
                                     start=(k == 0), stop=(k == C - 1))
                gsb = sb.tile([OUT_F, H], f32, tag="gsb")
                nc.any.tensor_copy(gsb[:], plp[:])
                nc.sync.dma_start(pool_io[0][:], gsb[:])
                if not _skip_coll:
                    nc.gpsimd.collective_compute(
                        "AllReduce", ALU.add, replica_groups=RG,
                        ins=[pool_io[0].opt()], outs=[pool_io[1].opt()])
                else:
                    nc.sync.dma_start(pool_io[1][:], pool_io[0][:])
                gg = sb.tile([OUT_F, H], f32, tag="gg")
                nc.sync.dma_start(gg[:], pool_io[1][:])

                gT = sb.tile([128, 2, OUT_F], bf16, tag="gT")
                for j in range(2):
                    tp = ps_t.tile([128, 128], f32, tag="tp")
                    nc.tensor.transpose(tp[:, 0:OUT_F],
                                        gg[0:OUT_F, j * 128:(j + 1) * 128],
                                        ident[0:OUT_F, 0:OUT_F])
                    nc.any.tensor_copy(gT[:, j, :], tp[:, 0:OUT_F])
                zps = ps_big.tile([128, H2], f32, tag="z1")
                for j in range(2):
                    nc.tensor.matmul(zps[0:OUT_F, 0:128], gT[:, j, :],
                                     mw1_sb[:, j, :],
                                     start=(j == 0), stop=(j == 1))
                zap = zps[0:OUT_F, 0:128]
                if flags.get("mlp_b1_rep"):
                    zb2 = sb.tile([OUT_F, 128], f32, tag="zb2")
                    nc.vector.tensor_add(zb2[:], zap, opt_sb["mlp_b1_rep"][:])
                    zap = zb2[:]
                zsb = sb.tile([OUT_F, 128], f32, tag="zsb")
                nc.scalar.activation(zsb[:], zap, AF.Relu)
                tp = ps_t.tile([128, 128], f32, tag="tp")
                nc.tensor.transpose(tp[:, 0:OUT_F], zsb[:], ident[0:OUT_F, 0:OUT_F])
                zT = sb.tile([128, OUT_F], bf16, tag="zT")
                nc.any.tensor_copy(zT[:], tp[:, 0:OUT_F])
                ops2 = ps_small.tile([OUT_F, OUT_F], f32, tag="small")
                nc.tensor.matmul(ops2[:], zT[:], mw2_sb[:], start=True, stop=True)
                oap = ops2[:]
                if flags.get("mlp_b2_rep"):
                    ob2 = sb.tile([OUT_F, OUT_F], f32, tag="ob2")
                    nc.vector.tensor_add(ob2[:], oap, opt_sb["mlp_b2_rep"][:])
                    oap = ob2[:]
                osb = sb.tile([OUT_F, OUT_F], f32, tag="osb")
                nc.any.tensor_copy(osb[:], oap)
                return nc.sync.dma_start(out_d[:], osb[:])

            _tail = None
            for _rep in range(repeat):
                _tail = one_pass(_tail, _rep)

    nc.compile()
    return nc, feeds, PER_CORE


def _prepare(inputs):
    x = np.asarray(inputs["x"], np.float32)
    edge_index = np.asarray(inputs["edge_index"], np.int32)
    batch = np.asarray(inputs["batch"], np.int32)

    x_t, src_idx, dstc, batchpos, T = _preprocess(x, edge_index, batch)

    from ml_dtypes import bfloat16 as np_bf16
    weights = {
        "x_t": x_t.astype(np_bf16),
        "src_idx": src_idx, "dstc": dstc, "batchpos": batchpos,
        "iota128": np.broadcast_to(
            np.arange(128, dtype=np.float16), (128, 128)).copy(),
        "iota64": np.broadcast_to(
            np.arange(NUM_G, dtype=np.float32), (128, NUM_G)).copy(),
        "enc_w": np.asarray(inputs["enc_w"], np.float32).astype(np_bf16),
        "mlp_w1": np.asarray(inputs["mlp_w1"], np.float32).astype(np_bf16),
        "mlp_w2": np.asarray(inputs["mlp_w2"], np.float32).astype(np_bf16),
        "t_vec": np.stack(
            [np.full(128, np.float32(inputs["l0_t"])),
             np.full(128, np.float32(inputs["l1_t"]))], axis=1),
        "ident": np.eye(128, dtype=np.float32),
    }
    for l in range(2):
        weights[f"w1_{l}"] = np.asarray(
            inputs[f"l{l}_w1"], np.float32).astype(np_bf16)
        weights[f"w2_{l}"] = np.asarray(
            inputs[f"l{l}_w2"], np.float32).astype(np_bf16)

    # generic-path (non-trivial bias/gain) tensors, replicated across rows
    flags = {}

    def opt(name, vec, trivial, rows):
        v = np.asarray(vec, np.float32)
        flags[name] = not np.allclose(v, trivial)
        if flags[name]:
            weights[name] = np.broadcast_to(v, (rows, v.shape[0])).copy()

    opt("enc_b_rep", inputs["enc_b"], 0.0, 128)
    for l in range(2):
        opt(f"b1_rep_{l}", inputs[f"l{l}_b1"], 0.0, 128)
        opt(f"lng_rep_{l}", inputs[f"l{l}_lng"], 1.0, 128)
        opt(f"lnb_rep_{l}", inputs[f"l{l}_lnb"], 0.0, 128)
        opt(f"b2_rep_{l}", inputs[f"l{l}_b2"], 0.0, 128)
        opt(f"ng_rep_{l}", inputs[f"l{l}_ng"], 1.0, 128)
        opt(f"nb_rep_{l}", inputs[f"l{l}_nb"], 0.0, 128)
    opt("mlp_b1_rep", inputs["mlp_b1"], 0.0, OUT_F)
    opt("mlp_b2_rep", inputs["mlp_b2"], 0.0, OUT_F)

    import os
    nc, feeds, PER_CORE = _build_program(
        T, flags, weights, repeat=int(os.environ.get("K_REPEAT", "1")))

    in_maps = []
    for c in range(NCORES):
        m = {}
        for name, arr in feeds.items():
            m[name] = np.ascontiguousarray(arr[c] if name in PER_CORE else arr)
        in_maps.append(m)

    return nc, in_maps


def kernel(**inputs):
    nc, in_maps = _prepare(inputs)
    from concourse import bass_utils
    res = bass_utils.run_bass_kernel_spmd(nc, in_maps,
                                          core_ids=list(range(NCORES)))
    return np.asarray(res.results[0]["out"], np.float32)


if __name__ == "__main__":
    import reference

    inputs = {k: np.asarray(v) for k, v in reference.setup_inputs().items()}
    out = kernel(**inputs)
    exp = np.asarray(reference.reference(**inputs))
    err = np.linalg.norm(out - exp) / np.linalg.norm(exp)
    print("Relative error:", err)



# revision 27
# speedup vs baseline: 1.0064x; 1.0064x over previous
"""DeeperGCN forward pass on 8 Trainium2 NeuronCores (Bass/Tile).

Strategy (graph-parallel, per sharding hint):
  - Nodes are binned into 8 cores x 20 chunks x 128 slots (125 real nodes per
    bin, degree-balanced via snake round-robin) so every chunk has ~2000
    incoming edges; edges live with their destination core.
  - Per GENConv layer each core streams RAW per-node h rows (fp16, 256 wide)
    into a pair-shared DRAM table as output chunks finish, AllGathers it
    (8x smaller than shipping em|me), then dma_gathers the src rows for its
    edges and computes m=relu(..)+eps, em=exp(t*m), me=m*em per edge on the
    consumer side.  One-hot matmuls segment-sum em|me into PSUM;
    agg = sum(me)/(sum(em)+1e-16) reproduces the softmax aggregation.
  - The layer-1 graph-LN is folded into the consumer: conv0 accumulates the
    stats while it runs, the 2-float AllReduce + scalar broadcast hide
    behind AllGather 1, and each gathered edge row is normalized with the
    global (mu, 1/std) before the exp.
  - One-hot tiles (agg + pool) are generated on device from packed dst/graph
    indices (is_equal against an iota tile, one DVE op per chunk).
  - Dense MLP (w1 -> LayerNorm -> relu -> w2) runs node-major per 128-node
    chunk in bf16 on the PE; global_add_pool is a one-hot matmul followed by
    a [64,256] AllReduce.
"""

import numpy as np

N_NODES = 20000
N_EDGES = 320000
F_IN = 128
H = 256
H2 = 512
OUT_F = 64
NUM_G = 64
NCORES = 8
C = 20            # chunks per core
BPC = 128         # node slots per chunk
NPB = 125         # real nodes per chunk (20000 / 160)
ROWS = C * BPC    # 2560 node slots per core
GROWS = NCORES * ROWS
EPS_MSG = 1e-7
LN_EPS = 1e-5
NTOT = float(N_NODES * H)


def _preprocess(x, edge_index, batch):
    """Bin nodes, build per-core gather indices / one-hot tiles."""
    src = edge_index[0].astype(np.int64)
    dst = edge_index[1].astype(np.int64)
    deg = np.bincount(dst, minlength=N_NODES)

    # snake round-robin of degree-sorted nodes over 160 bins -> 125 nodes/bin
    order = np.argsort(-deg, kind="stable")
    nbins = NCORES * C
    node_core = np.empty(N_NODES, np.int64)
    node_chunk = np.empty(N_NODES, np.int64)
    node_pos = np.empty(N_NODES, np.int64)
    rounds = N_NODES // nbins  # 125
    idx = np.arange(N_NODES)
    rnd = idx // nbins
    pos_in_round = idx % nbins
    binidx = np.where(rnd % 2 == 0, pos_in_round, nbins - 1 - pos_in_round)
    node_core[order] = binidx % NCORES
    node_chunk[order] = binidx // NCORES
    node_pos[order] = rnd
    assert rounds == NPB

    # slot id of every node in the AllGather'ed table
    slot = node_core * ROWS + node_chunk * BPC + node_pos  # < 20480
    assert slot.max() < 2 ** 15

    # tiles per chunk
    e_core = node_core[dst]
    e_chunk = node_chunk[dst]
    bin_of_edge = e_core * C + e_chunk
    bin_edges = np.bincount(bin_of_edge, minlength=nbins)
    T = int(np.ceil(bin_edges.max() / 128))
    EPC = T * 128

    # x fed pre-transposed: xT[c] = [F_IN, ROWS] so encoder needs no PE transpose
    x_sh = np.zeros((NCORES, ROWS, F_IN), np.float32)
    rows_all = node_chunk * BPC + node_pos
    for c in range(NCORES):
        m = node_core == c
        x_sh[c, rows_all[m]] = x[m]
    x_t = np.ascontiguousarray(x_sh.transpose(0, 2, 1))  # [NC, F_IN, ROWS]

    # per-node graph id (for on-device pool one-hot); -1 for empty slots
    batchpos = np.full((NCORES, ROWS), -1.0, np.float32)
    b = batch.astype(np.int64)
    for c in range(NCORES):
        m = node_core == c
        batchpos[c, rows_all[m]] = b[m].astype(np.float32)
    # [NC, 128, C] layout: batchpos[c][p, k] = graph of node (chunk k, pos p)
    batchpos = np.ascontiguousarray(
        batchpos.reshape(NCORES, C, BPC).transpose(0, 2, 1))

    # per (core, chunk): edge src slots + one-hot dst columns
    src_slots = np.zeros((NCORES, C, EPC), np.int16)
    dst_cols = np.full((NCORES, C, EPC), -1, np.int64)
    order_e = np.lexsort((dst, bin_of_edge))
    s_sorted = src[order_e]
    d_sorted = dst[order_e]
    bin_sorted = bin_of_edge[order_e]
    starts = np.searchsorted(bin_sorted, np.arange(nbins))
    ends = np.searchsorted(bin_sorted, np.arange(nbins) + 1)
    for bi in range(nbins):
        c, k = bi // C, bi % C
        lo, hi = starts[bi], ends[bi]
        n = hi - lo
        src_slots[c, k, :n] = slot[s_sorted[lo:hi]].astype(np.int16)
        dst_cols[c, k, :n] = node_pos[d_sorted[lo:hi]]

    # wrapped int16 index layout: element i of a chunk at [i % 16, i // 16];
    # fed as 16 partitions, replicated to 128 on device
    src_idx = (
        src_slots.reshape(NCORES, C, EPC // 16, 16)
        .transpose(0, 1, 3, 2)            # [NC, C, 16, EPC//16]
        .transpose(0, 2, 1, 3)            # [NC, 16, C, EPC//16]
        .reshape(NCORES, 16, C * EPC // 16).copy()
    )

    # packed dst position per edge for on-device one-hot generation:
    # dstc[c][p, k*T + t] = dst position (0..127) of edge t*128+p of chunk k
    dstc = (dst_cols.astype(np.float32)
            .reshape(NCORES, C, T, 128)
            .transpose(0, 3, 1, 2)        # [NC, 128, C, T]
            .reshape(NCORES, 128, C * T)
            .astype(np.float16).copy())

    return x_t, src_idx, dstc, batchpos, T


def _build_program(T, flags, weights, repeat=1):
    import concourse.bacc as bacc
    import concourse.bass as bass
    import concourse.mybir as mybir
    import concourse.tile as tile
    from concourse import library_config

    f32 = mybir.dt.float32
    bf16 = mybir.dt.bfloat16
    f16 = mybir.dt.float16
    i16 = mybir.dt.int16
    AF = mybir.ActivationFunctionType
    ALU = mybir.AluOpType
    EPC = T * 128
    RG = [list(range(NCORES))]

    import os
    _skip_coll = bool(int(os.environ.get("K_SKIP_COLL", "0")))
    _skip_gath = bool(int(os.environ.get("K_SKIP_GATH", "0")))
    _nq = int(os.environ.get("K_NQUEUES", "4"))
    _shared_ag = bool(int(os.environ.get("K_SHARED", "1")))
    nc = bacc.Bacc("TRN2", target_bir_lowering=False, debug=False,
                   num_devices=NCORES, num_swdge_queues=_nq)

    feeds = {}
    PER_CORE = {"x_t", "src_idx", "dstc", "batchpos"}

    def inp(name, arr, dtype):
        """Declare an input fed per-core (arr: [NCORES, ...]) or replicated."""
        feeds[name] = arr
        shape = list(arr.shape[1:]) if name in PER_CORE else list(arr.shape)
        return nc.dram_tensor(name, shape, dtype, kind="ExternalInput")

    x_in = inp("x_t", weights["x_t"], bf16)
    srcidx_d = inp("src_idx", weights["src_idx"], i16)
    dstc_d = inp("dstc", weights["dstc"], f16)
    batchpos_d = inp("batchpos", weights["batchpos"], f32)
    iota128_d = inp("iota128", weights["iota128"], f16)
    iota64_d = inp("iota64", weights["iota64"], f32)
    encw_d = inp("enc_w", weights["enc_w"], bf16)
    w1_d = [inp(f"w1_{l}", weights[f"w1_{l}"], bf16) for l in range(2)]
    w2_d = [inp(f"w2_{l}", weights[f"w2_{l}"], bf16) for l in range(2)]
    mw1_d = inp("mlp_w1", weights["mlp_w1"], bf16)
    mw2_d = inp("mlp_w2", weights["mlp_w2"], bf16)
    tvec_d = inp("t_vec", weights["t_vec"], f32)  # [128, 2]
    ident_d = inp("ident", weights["ident"], f32)  # [128, 128] identity
    out_d = nc.dram_tensor("out", [OUT_F, OUT_F], f32, kind="ExternalOutput")

    # optional generic-path tensors (replicated rows), only when non-trivial
    opt_d = {}
    for nm in flags:
        if flags[nm]:
            opt_d[nm] = inp(nm, weights[nm], f32)

    with tile.TileContext(nc) as tc:
        from contextlib import ExitStack

        with ExitStack() as ctx:
            cpool = ctx.enter_context(tc.tile_pool(name="const", bufs=1))
            wpool = ctx.enter_context(tc.tile_pool(name="wts", bufs=1))
            hpool = ctx.enter_context(tc.tile_pool(name="hbuf", bufs=1))
            sb = ctx.enter_context(tc.tile_pool(name="work", bufs=2))
            sb2 = ctx.enter_context(tc.tile_pool(name="work2", bufs=2))
            gpool = ctx.enter_context(tc.tile_pool(name="gather", bufs=7))
            empool = ctx.enter_context(tc.tile_pool(name="emme", bufs=2))
            ohpool = ctx.enter_context(tc.tile_pool(name="onehot", bufs=3))
            pk_pool = ctx.enter_context(tc.tile_pool(name="pack", bufs=2))
            ps_t = ctx.enter_context(
                tc.tile_pool(name="ps_t", bufs=2, space="PSUM"))
            ps_agg = ctx.enter_context(
                tc.tile_pool(name="ps_agg", bufs=3, space="PSUM"))
            ps_big = ctx.enter_context(
                tc.tile_pool(name="ps_big", bufs=2, space="PSUM"))
            ps_small = ctx.enter_context(
                tc.tile_pool(name="ps_small", bufs=1, space="PSUM"))
            dram = ctx.enter_context(
                tc.tile_pool(name="dram", bufs=1, space="DRAM"))

            # ---- constants & weights in SBUF ----
            nc.gpsimd.load_library(library_config.mlp)
            ident = cpool.tile([128, 128], f32)
            nc.sync.dma_start(ident[:], ident_d[:])
            ones_col = cpool.tile([128, 1], f32)
            nc.vector.memset(ones_col[:], 1.0)
            ones_row = cpool.tile([1, 128], f32)
            nc.vector.memset(ones_row[:], 1.0)
            epsln_col = cpool.tile([128, 1], f32)
            nc.vector.memset(epsln_col[:], LN_EPS)

            encw_sb = wpool.tile([128, H], bf16)
            nc.sync.dma_start(encw_sb[:], encw_d[:])
            w1_sb, w2_sb = [], []
            for l in range(2):
                a = wpool.tile([128, 2, H2], bf16, tag=f"w1sb{l}")
                nc.sync.dma_start(
                    a[:], w1_d[l].ap().rearrange("(j p) n -> p j n", p=128))
                w1_sb.append(a)
                bq = wpool.tile([128, 4, H], bf16, tag=f"w2sb{l}")
                nc.sync.dma_start(
                    bq[:], w2_d[l].ap().rearrange("(j p) n -> p j n", p=128))
                w2_sb.append(bq)
            mw1_sb = wpool.tile([128, 2, 128], bf16)
            nc.sync.dma_start(
                mw1_sb[:], mw1_d.ap().rearrange("(j p) n -> p j n", p=128))
            mw2_sb = wpool.tile([128, OUT_F], bf16)
            nc.sync.dma_start(mw2_sb[:], mw2_d[:])
            srcidx_sb = wpool.tile([128, C * EPC // 16], i16)
            for r in range(8):
                nc.sync.dma_start(srcidx_sb[r * 16:(r + 1) * 16, :], srcidx_d[:])
            xT_sb = wpool.tile([128, ROWS], bf16)
            nc.sync.dma_start(xT_sb[:], x_in[:])
            dstc_sb = wpool.tile([128, C * T], f16)
            nc.sync.dma_start(dstc_sb[:], dstc_d[:])
            batchpos_sb = wpool.tile([128, C], f32)
            nc.sync.dma_start(batchpos_sb[:], batchpos_d[:])
            iota128_sb = wpool.tile([128, 128], f16)
            nc.sync.dma_start(iota128_sb[:], iota128_d[:])
            iotaT_sb = wpool.tile([128, T, 128], f16)
            for _t in range(T):
                nc.vector.tensor_copy(iotaT_sb[:, _t, :], iota128_sb[:])
            iota64_sb = wpool.tile([128, NUM_G], f32)
            nc.sync.dma_start(iota64_sb[:], iota64_d[:])
            tvec_sb = wpool.tile([128, 2], f32)
            nc.sync.dma_start(tvec_sb[:], tvec_d[:])
            opt_sb = {}
            for nm, d in opt_d.items():
                tl = wpool.tile(list(d.shape), f32, tag=f"opt_{nm}")
                nc.sync.dma_start(tl[:], d[:])
                opt_sb[nm] = tl


            # ---- DRAM comm buffers (ag bufs per-rep: Shared DRAM wants a
            # single writer per buffer) ----
            _ag_space = "Shared" if _shared_ag else "Local"
            red_io = [dram.tile([1, 16], f32, tag=f"red{i}",
                                name=f"red{i}") for i in range(4)]
            pool_io = [dram.tile([OUT_F, H], f32, tag=f"pio{i}",
                                 name=f"pio{i}") for i in range(2)]

            from concourse.tile_rust import add_dep_helper

            def one_pass(prev_tail, rep):
                ag_in = [dram.tile([ROWS, H], f16, tag=f"ag_in{l}_r{rep}",
                                   name=f"ag_in{l}_r{rep}") for l in range(2)]
                ag_out = [dram.tile([GROWS, H], f16, tag=f"ag_out{l}_r{rep}",
                                    name=f"ag_out{l}_r{rep}",
                                    addr_space=_ag_space)
                          for l in range(2)]
                # per-rep activations (tags reuse slots across reps)
                h_enc = hpool.tile([128, C, H], f32, tag="h_enc",
                                   name="h_enc")
                h1 = hpool.tile([128, C, H], f32, tag="h1", name="h1")
                r_buf = hpool.tile([128, C, H], f32, tag="h_enc",
                                   name="r_buf")  # h_enc dead after conv0
                h2 = hpool.tile([128, C, H], f32, tag="h2", name="h2")
                # ================= encoder =================
                # table rows (raw h in fp16) stream to ag_in[0] as chunks
                # finish, so AllGather 0 can start right at encoder end
                for k in range(C):
                    hps = ps_big.tile([128, H2], f32, tag="z1")
                    _mm = nc.tensor.matmul(
                        hps[:, 0:H],
                        xT_sb[:, k * BPC:(k + 1) * BPC],
                        encw_sb[:],
                        start=True, stop=True)
                    if k == 0 and prev_tail is not None:
                        add_dep_helper(prev_tail.ins, _mm.ins, True,
                                       "serialize reps")
                    if flags.get("enc_b_rep"):
                        nc.vector.tensor_add(h_enc[:, k, :], hps[:, 0:H],
                                             opt_sb["enc_b_rep"][:])
                    else:
                        nc.any.tensor_copy(h_enc[:, k, :], hps[:, 0:H])
                    pk = pk_pool.tile([128, H], f16, tag="pk")
                    nc.any.tensor_copy(pk[:], h_enc[:, k, :])
                    nc.sync.dma_start(ag_in[0][k * BPC:(k + 1) * BPC, :], pk[:])

                # ================= helpers =================
                def issue_ag(l):
                    if not _skip_coll:
                        nc.gpsimd.collective_compute(
                            "AllGather", mybir.AluOpType.bypass,
                            replica_groups=RG,
                            ins=[ag_in[l].opt()], outs=[ag_out[l].opt()])
                    else:
                        nc.sync.dma_start(ag_out[l][0:ROWS, :], ag_in[l][:, :])

                def conv(l, x_t, res_t, out_t, ln_bc=None, next_agin=None,
                         next_stats=None):
                    """One GENConv layer. Table rows are RAW h values (fp16);
                    per gathered edge tile this computes
                      m  = relu((h - mu)*inv) + eps   (ln_bc) or relu(h)+eps
                      em = exp(t*m);  me = m*em
                    If next_agin is set, h_out rows stream to it (raw fp16)
                    as chunks finish; next_stats accumulates graph-LN stats.
                    """
                    TH = T // 2
                    for k in range(C):
                        emme = empool.tile([128, T, H2], f16, tag="emme",
                                           name="emme")
                        for h in range(2):
                            gt = gpool.tile([128, TH, H], f16, tag="gt",
                                            name="gt")
                            if not _skip_gath:
                                off = k * (EPC // 16) + h * (EPC // 32)
                                nc.gpsimd.dma_gather(
                                    gt[:], ag_out[l][:, :],
                                    srcidx_sb[:, off:off + EPC // 32],
                                    EPC // 2, EPC // 2, H,
                                    single_packet=False,
                                    queue_num=(2 * k + h) % _nq)
                            else:
                                nc.vector.memset(gt[:, 0, 0:4], 0.0)
                            if ln_bc is not None:
                                base = sb2.tile([128, TH, H], f16, tag="nrm")
                                nc.vector.tensor_scalar(
                                    base[:], gt[:], ln_bc[:, 0:1],
                                    ln_bc[:, 1:2], ALU.subtract, ALU.mult)
                                if flags.get(f"ng_rep_{l}"):
                                    nc.vector.tensor_mul(
                                        base[:], base[:],
                                        opt_sb[f"ng_rep_{l}"][:]
                                        .unsqueeze(1).broadcast_to([128, TH, H]))
                                if flags.get(f"nb_rep_{l}"):
                                    nc.vector.tensor_add(
                                        base[:], base[:],
                                        opt_sb[f"nb_rep_{l}"][:]
                                        .unsqueeze(1).broadcast_to([128, TH, H]))
                            else:
                                base = gt
                            mt = sb2.tile([128, TH, H], f16, tag="mt")
                            nc.vector.tensor_scalar(mt[:], base[:], 0.0,
                                                    EPS_MSG, ALU.max, ALU.add)
                            sl = emme[:, h * TH:(h + 1) * TH, :]
                            nc.scalar.activation(sl[:, :, 0:H], mt[:],
                                                 AF.Exp,
                                                 scale=tvec_sb[:, l:l + 1])
                            nc.vector.tensor_mul(sl[:, :, H:H2], mt[:],
                                                 sl[:, :, 0:H])
                        oh = ohpool.tile([128, T, 128], f16, tag="oh")
                        nc.vector.tensor_tensor(
                            oh[:], iotaT_sb[:],
                            dstc_sb[:, k * T:(k + 1) * T]
                            .unsqueeze(2).broadcast_to([128, T, 128]),
                            ALU.is_equal)
                        agg = ps_agg.tile([128, H2], f32, tag="agg")
                        for t in range(T):
                            nc.tensor.matmul(agg[:], oh[:, t, :],
                                             emme[:, t, :],
                                             start=(t == 0), stop=(t == T - 1))
                        den = sb.tile([128, H], f32, tag="den")
                        nc.vector.tensor_scalar_add(den[:], agg[:, 0:H], 1e-16)
                        rec = sb.tile([128, H], f32, tag="rec")
                        nc.vector.reciprocal(rec[:], den[:])
                        hin = sb.tile([128, H], f32, tag="hin")
                        nc.vector.tensor_mul(hin[:], agg[:, H:H2], rec[:])
                        nc.vector.tensor_add(hin[:], hin[:], x_t[:, k, :])
                        hinT = sb.tile([128, 2, 128], bf16, tag="hinT")
                        for j in range(2):
                            tp = ps_t.tile([128, 128], f32, tag="tp")
                            nc.tensor.transpose(
                                tp[:], hin[:, j * 128:(j + 1) * 128], ident[:])
                            nc.any.tensor_copy(hinT[:, j, :], tp[:])
                        z1 = ps_big.tile([128, H2], f32, tag="z1")
                        for j in range(2):
                            nc.tensor.matmul(z1[:], hinT[:, j, :],
                                             w1_sb[l][:, j, :],
                                             start=(j == 0), stop=(j == 1))
                        if flags.get(f"b1_rep_{l}"):
                            zb = sb2.tile([128, H2], f32, tag="zb")
                            nc.vector.tensor_add(zb[:], z1[:],
                                                 opt_sb[f"b1_rep_{l}"][:])
                            z1s = zb
                        else:
                            z1s = z1
                        st6 = sb.tile([128, 6], f32, tag="st6")
                        nc.vector.bn_stats(st6[:], z1s[:])
                        mv = sb.tile([128, 2], f32, tag="mv")
                        nc.vector.bn_aggr(mv[:], st6[:])
                        sd = sb.tile([128, 1], f32, tag="sd")
                        nc.scalar.activation(sd[:], mv[:, 1:2], AF.Sqrt,
                                             bias=epsln_col[:])
                        rs = sb.tile([128, 1], f32, tag="rs")
                        nc.vector.reciprocal(rs[:], sd[:])
                        h2c = sb2.tile([128, H2], f32, tag="h2c")
                        nc.vector.tensor_scalar(h2c[:], z1s[:], mv[:, 0:1], rs[:],
                                                ALU.subtract, ALU.mult)
                        if flags.get(f"lng_rep_{l}"):
                            nc.vector.tensor_mul(h2c[:], h2c[:],
                                                 opt_sb[f"lng_rep_{l}"][:])
                        if flags.get(f"lnb_rep_{l}"):
                            nc.vector.tensor_add(h2c[:], h2c[:],
                                                 opt_sb[f"lnb_rep_{l}"][:])
                        nc.scalar.activation(h2c[:], h2c[:], AF.Relu)
                        h2T = sb2.tile([128, 4, 128], bf16, tag="h2T")
                        for j in range(4):
                            tp = ps_t.tile([128, 128], f32, tag="tp")
                            nc.tensor.transpose(
                                tp[:], h2c[:, j * 128:(j + 1) * 128], ident[:])
                            nc.any.tensor_copy(h2T[:, j, :], tp[:])
                        ops = ps_big.tile([128, H2], f32, tag="z1")
                        for j in range(4):
                            nc.tensor.matmul(ops[:, 0:H], h2T[:, j, :],
                                             w2_sb[l][:, j, :],
                                             start=(j == 0), stop=(j == 3))
                        src_ap = ops[:, 0:H]
                        if flags.get(f"b2_rep_{l}"):
                            ob = sb.tile([128, H], f32, tag="ob")
                            nc.vector.tensor_add(ob[:], src_ap,
                                                 opt_sb[f"b2_rep_{l}"][:])
                            src_ap = ob[:]
                        if res_t is not None:
                            nc.vector.tensor_add(out_t[:, k, :], src_ap,
                                                 res_t[:, k, :])
                        else:
                            nc.any.tensor_copy(out_t[:, k, :], src_ap)
                        if next_agin is not None:
                            pk = pk_pool.tile([128, H], f16, tag="pk")
                            nc.any.tensor_copy(pk[:], out_t[:, k, :])
                            nc.sync.dma_start(
                                next_agin[k * BPC:(k + 1) * BPC, :], pk[:])
                        if next_stats is not None:
                            nc.vector.tensor_reduce(
                                next_stats[0:NPB, 0, k:k + 1],
                                out_t[0:NPB, k, :],
                                mybir.AxisListType.X, ALU.add)
                            scr = sb2.tile([128, H], f32, tag="scr")
                            nc.scalar.activation(
                                scr[0:NPB, :], out_t[0:NPB, k, :], AF.Square,
                                accum_out=next_stats[0:NPB, 1, k:k + 1])

                def ln_reduce(stats, io_base):
                    """stats [128,2,C] -> AllReduce -> bc [128,2] = [mu, inv]"""
                    cps = ps_t.tile([1, 2 * C], f32, tag="tp")
                    nc.tensor.matmul(cps[:], ones_col[0:NPB, :],
                                     stats[0:NPB, :, :],
                                     start=True, stop=True)
                    tot = sb.tile([1, 16], f32, tag="tot")
                    nc.vector.memset(tot[:], 0.0)
                    nc.vector.tensor_reduce(
                        tot[0:1, 0:2],
                        cps[0:1, :].rearrange("p (s k) -> p s k", k=C),
                        mybir.AxisListType.X, ALU.add)
                    nc.sync.dma_start(red_io[io_base][:], tot[:])
                    if not _skip_coll:
                        nc.gpsimd.collective_compute(
                            "AllReduce", ALU.add, replica_groups=RG,
                            ins=[red_io[io_base].opt()],
                            outs=[red_io[io_base + 1].opt()])
                    else:
                        nc.sync.dma_start(red_io[io_base + 1][:],
                                          red_io[io_base][:])
                    gt2 = sb.tile([1, 16], f32, tag="gt2")
                    nc.sync.dma_start(gt2[:], red_io[io_base + 1][:])
                    mu = sb.tile([1, 4], f32, tag="mu")
                    nc.vector.tensor_scalar_mul(mu[0:1, 0:2], gt2[0:1, 0:2],
                                                1.0 / NTOT)  # [mean, E[x^2]]
                    nc.vector.tensor_mul(mu[0:1, 2:3], mu[0:1, 0:1], mu[0:1, 0:1])
                    nc.vector.tensor_sub(mu[0:1, 2:3], mu[0:1, 1:2], mu[0:1, 2:3])
                    sdg = sb.tile([1, 1], f32, tag="sdg")
                    nc.scalar.activation(sdg[:], mu[0:1, 2:3], AF.Sqrt)
                    nc.vector.tensor_scalar_add(sdg[:], sdg[:], LN_EPS)
                    pair = sb.tile([1, 2], f32, tag="pair")
                    nc.any.tensor_copy(pair[0:1, 0:1], mu[0:1, 0:1])
                    nc.vector.reciprocal(pair[0:1, 1:2], sdg[:])
                    bcp = ps_t.tile([128, 2], f32, tag="tp")
                    nc.tensor.matmul(bcp[:], ones_row[:], pair[:],
                                     start=True, stop=True)
                    bc = sb.tile([128, 2], f32, tag="bc")
                    nc.any.tensor_copy(bc[:], bcp[:])
                    return bc

                def ln_apply(src_t, dst_t, bc, pref):
                    for k in range(C):
                        nc.vector.tensor_scalar(dst_t[:, k, :], src_t[:, k, :],
                                                bc[:, 0:1], bc[:, 1:2],
                                                ALU.subtract, ALU.mult)
                        if flags.get(f"ng_rep_{pref}"):
                            nc.vector.tensor_mul(dst_t[:, k, :], dst_t[:, k, :],
                                                 opt_sb[f"ng_rep_{pref}"][:])
                        if flags.get(f"nb_rep_{pref}"):
                            nc.vector.tensor_add(dst_t[:, k, :], dst_t[:, k, :],
                                                 opt_sb[f"nb_rep_{pref}"][:])
                        nc.scalar.activation(dst_t[:, k, :], dst_t[:, k, :],
                                             AF.Relu)

                # ================= network =================
                # chain: enc -> AG0 -> conv0 -> [AR(ln1) || AG1] -> conv1
                #        -> AR(ln0) -> pool -> AR(pool) -> MLP
                # conv0 streams raw h1 rows to ag_in[1] and accumulates ln1
                # stats as it goes; the ln1 AllReduce + bc + local r_buf all
                # hide behind AllGather 1.
                issue_ag(0)
                stats1 = sb2.tile([128, 2, C], f32, tag="stats",
                                  name="stats1")
                conv(0, h_enc, None, h1, next_agin=ag_in[1],
                     next_stats=stats1)
                bc1 = ln_reduce(stats1, 0)         # layers[1] norm scalars
                issue_ag(1)
                ln_apply(h1, r_buf, bc1, 1)        # local residual input
                stats0 = sb2.tile([128, 2, C], f32, tag="stats",
                                  name="stats0")
                conv(1, r_buf, h1, h2, ln_bc=bc1, next_stats=stats0)
                bc0 = ln_reduce(stats0, 2)         # layers[0] norm scalars
                ln_apply(h2, h2, bc0, 0)

                # ================= pool + final MLP =================
                plp = ps_small.tile([OUT_F, H], f32, tag="small")
                for k in range(C):
                    oh64 = sb.tile([128, NUM_G], f32, tag="oh64")
                    nc.vector.tensor_scalar(
                        oh64[:], iota64_sb[:], batchpos_sb[:, k:k + 1], None,
                        ALU.is_equal)
                    nc.tensor.matmul(plp[:], oh64[:], h2[:, k, :],
                                     start=(k == 0), stop=(k == C - 1))
                gsb = sb.tile([OUT_F, H], f32, tag="gsb")
                nc.any.tensor_copy(gsb[:], plp[:])
                nc.sync.dma_start(pool_io[0][:], gsb[:])
                if not _skip_coll:
                    nc.gpsimd.collective_compute(
                        "AllReduce", ALU.add, replica_groups=RG,
                        ins=[pool_io[0].opt()], outs=[pool_io[1].opt()])
                else:
                    nc.sync.dma_start(pool_io[1][:], pool_io[0][:])
                gg = sb.tile([OUT_F, H], f32, tag="gg")
                nc.sync.dma_start(gg[:], pool_io[1][:])

                gT = sb.tile([128, 2, OUT_F], bf16, tag="gT")
                for j in range(2):
                    tp = ps_t.tile([128, 128], f32, tag="tp")
                    nc.tensor.transpose(tp[:, 0:OUT_F],
                                        gg[0:OUT_F, j * 128:(j + 1) * 128],
                                        ident[0:OUT_F, 0:OUT_F])
                    nc.any.tensor_copy(gT[:, j, :], tp[:, 0:OUT_F])
                zps = ps_big.tile([128, H2], f32, tag="z1")
                for j in range(2):
                    nc.tensor.matmul(zps[0:OUT_F, 0:128], gT[:, j, :],
                                     mw1_sb[:, j, :],
                                     start=(j == 0), stop=(j == 1))
                zap = zps[0:OUT_F, 0:128]
                if flags.get("mlp_b1_rep"):
                    zb2 = sb.tile([OUT_F, 128], f32, tag="zb2")
                    nc.vector.tensor_add(zb2[:], zap, opt_sb["mlp_b1_rep"][:])
                    zap = zb2[:]
                zsb = sb.tile([OUT_F, 128], f32, tag="zsb")
                nc.scalar.activation(zsb[:], zap, AF.Relu)
                tp = ps_t.tile([128, 128], f32, tag="tp")
                nc.tensor.transpose(tp[:, 0:OUT_F], zsb[:], ident[0:OUT_F, 0:OUT_F])
                zT = sb.tile([128, OUT_F], bf16, tag="zT")
                nc.any.tensor_copy(zT[:], tp[:, 0:OUT_F])
                ops2 = ps_small.tile([OUT_F, OUT_F], f32, tag="small")
                nc.tensor.matmul(ops2[:], zT[:], mw2_sb[:], start=True, stop=True)
                oap = ops2[:]
                if flags.get("mlp_b2_rep"):
                    ob2 = sb.tile([OUT_F, OUT_F], f32, tag="ob2")
                    nc.vector.tensor_add(ob2[:], oap, opt_sb["mlp_b2_rep"][:])
                    oap = ob2[:]
                osb = sb.tile([OUT_F, OUT_F], f32, tag="osb")
                nc.any.tensor_copy(osb[:], oap)
                return nc.sync.dma_start(out_d[:], osb[:])

            _tail = None
            for _rep in range(repeat):
                _tail = one_pass(_tail, _rep)

    nc.compile()
    return nc, feeds, PER_CORE


def _prepare(inputs):
    x = np.asarray(inputs["x"], np.float32)
    edge_index = np.asarray(inputs["edge_index"], np.int32)
    batch = np.asarray(inputs["batch"], np.int32)

    x_t, src_idx, dstc, batchpos, T = _preprocess(x, edge_index, batch)

    from ml_dtypes import bfloat16 as np_bf16
    weights = {
        "x_t": x_t.astype(np_bf16),
        "src_idx": src_idx, "dstc": dstc, "batchpos": batchpos,
        "iota128": np.broadcast_to(
            np.arange(128, dtype=np.float16), (128, 128)).copy(),
        "iota64": np.broadcast_to(
            np.arange(NUM_G, dtype=np.float32), (128, NUM_G)).copy(),
        "enc_w": np.asarray(inputs["enc_w"], np.float32).astype(np_bf16),
        "mlp_w1": np.asarray(inputs["mlp_w1"], np.float32).astype(np_bf16),
        "mlp_w2": np.asarray(inputs["mlp_w2"], np.float32).astype(np_bf16),
        "t_vec": np.stack(
            [np.full(128, np.float32(inputs["l0_t"])),
             np.full(128, np.float32(inputs["l1_t"]))], axis=1),
        "ident": np.eye(128, dtype=np.float32),
    }
    for l in range(2):
        weights[f"w1_{l}"] = np.asarray(
            inputs[f"l{l}_w1"], np.float32).astype(np_bf16)
        weights[f"w2_{l}"] = np.asarray(
            inputs[f"l{l}_w2"], np.float32).astype(np_bf16)

    # generic-path (non-trivial bias/gain) tensors, replicated across rows
    flags = {}

    def opt(name, vec, trivial, rows):
        v = np.asarray(vec, np.float32)
        flags[name] = not np.allclose(v, trivial)
        if flags[name]:
            weights[name] = np.broadcast_to(v, (rows, v.shape[0])).copy()

    opt("enc_b_rep", inputs["enc_b"], 0.0, 128)
    for l in range(2):
        opt(f"b1_rep_{l}", inputs[f"l{l}_b1"], 0.0, 128)
        opt(f"lng_rep_{l}", inputs[f"l{l}_lng"], 1.0, 128)
        opt(f"lnb_rep_{l}", inputs[f"l{l}_lnb"], 0.0, 128)
        opt(f"b2_rep_{l}", inputs[f"l{l}_b2"], 0.0, 128)
        opt(f"ng_rep_{l}", inputs[f"l{l}_ng"], 1.0, 128)
        opt(f"nb_rep_{l}", inputs[f"l{l}_nb"], 0.0, 128)
    opt("mlp_b1_rep", inputs["mlp_b1"], 0.0, OUT_F)
    opt("mlp_b2_rep", inputs["mlp_b2"], 0.0, OUT_F)

    import os
    nc, feeds, PER_CORE = _build_program(
        T, flags, weights, repeat=int(os.environ.get("K_REPEAT", "1")))

    in_maps = []
    for c in range(NCORES):
        m = {}
        for name, arr in feeds.items():
            m[name] = np.ascontiguousarray(arr[c] if name in PER_CORE else arr)
        in_maps.append(m)

    return nc, in_maps


def kernel(**inputs):
    nc, in_maps = _prepare(inputs)
    from concourse import bass_utils
    res = bass_utils.run_bass_kernel_spmd(nc, in_maps,
                                          core_ids=list(range(NCORES)))
    return np.asarray(res.results[0]["out"], np.float32)


if __name__ == "__main__":
    import reference

    inputs = {k: np.asarray(v) for k, v in reference.setup_inputs().items()}
    out = kernel(**inputs)
    exp = np.asarray(reference.reference(**inputs))
    err = np.linalg.norm(out - exp) / np.linalg.norm(exp)
    print("Relative error:", err)

